# revision 14
# baseline (speedup 1.0000x reference)
"""Cosine-similarity MoE routing kernel for Trainium2 (8 NeuronCores, SPMD).

reference:
    dots = z @ ek.T                      # [N, E]
    sim  = dots / max(||z|| * ||ek||, 1e-8)
    weighted = softmax(sim, -1) @ ek     # [N, D]
    returns (sim, weighted)

Sharding: data-parallel over tokens (N) across 8 cores; expert_keys replicated.

v2 design notes:
  - All matmuls in float32r (TF32-like, 1 cyc/col at N>=512; ~1.6e-4 rel err).
  - matmul1 flipped: lhsT = ekT_scaled chunk [128d, 64] stationary, rhs = zT
    [128d, 512 tokens] moving -> dotsT [64, 512] psum; one PSUM bank; 16-chunk
    accumulation; processed in 4-tile groups of 512 tokens.
  - z enters PE as float32r (bitcast, no cast cost); transposes at 1.5 cyc/col.
  - ScalarE stays in ONE table set (natural_log_exp_and_others): Square, Ln,
    Exp, Copy.  1/sqrt(x) = exp(-0.5*ln(x)); no Sqrt, no Reciprocal on ACT.
  - PSUM->SBUF evictions split between ScalarE (zT) and VectorE (weighted).
"""

from contextlib import ExitStack

import numpy as np

import concourse.bass as bass
import concourse.mybir as mybir
import concourse.tile as tile
from concourse.bass_utils import run_bass_kernel_spmd
from concourse.masks import make_identity

F32 = mybir.dt.float32
F32R = mybir.dt.float32r
BF16 = mybir.dt.bfloat16
AF = mybir.ActivationFunctionType

N_CORES = 8
N_FULL, D, E = 32768, 2048, 64
N_SHARD = N_FULL // N_CORES  # 4096
P = 128
G = 4                        # tiles per group (512 tokens)
N_GROUPS = N_SHARD // (G * P)  # 8
D_CHUNKS = D // P            # 16


def _split_multi_waits(nc, max_waits=1):
    """This walrus build allows very few sem-waits per instruction; split the
    extras into same-engine NOPs placed immediately before the instruction."""
    n = 0
    for func in nc.m.functions:
        for block in func.blocks:
            out = []
            for inst in block.instructions:
                si = inst.sync_info
                waits = list(si.on_wait) if (si is not None and si.on_wait) else []
                if len(waits) > max_waits:
                    head, keep = waits[:-max_waits], waits[-max_waits:]
                    for j, w in enumerate(head):
                        out.append(
                            mybir.InstNoOp(
                                name=f"{inst.name}_wsplit{j}",
                                engine=inst.engine,
                                sync_info=mybir.SyncInfo(on_wait=[w], on_update=[]),
                                bass_nofuse=True,
                            )
                        )
                        n += 1
                    si.on_wait = keep
                out.append(inst)
            block.instructions[:] = out
    return n


def _build():
    nc = bass.Bass(trn_type="TRN2")
    z = nc.dram_tensor("z", [N_SHARD, D], F32, kind="ExternalInput")
    ek = nc.dram_tensor("expert_keys", [E, D], F32, kind="ExternalInput")
    sim_out = nc.dram_tensor("similarity", [N_SHARD, E], F32, kind="ExternalOutput")
    w_out = nc.dram_tensor("weighted", [N_SHARD, D], F32, kind="ExternalOutput")

    with tile.TileContext(nc) as tc, ExitStack() as ctx:
        singles = ctx.enter_context(tc.tile_pool(name="singles", bufs=1))
        zpool = ctx.enter_context(tc.tile_pool(name="zpool", bufs=3))
        ztpool = ctx.enter_context(tc.tile_pool(name="ztpool", bufs=2))
        wpool = ctx.enter_context(tc.tile_pool(name="wpool", bufs=2))
        smalls = ctx.enter_context(tc.tile_pool(name="smalls", bufs=8))
        dtpool = ctx.enter_context(tc.tile_pool(name="dtpool", bufs=2))
        ps_zt = ctx.enter_context(tc.tile_pool(name="ps_zt", bufs=2, space="PSUM"))
        ps_dt = ctx.enter_context(tc.tile_pool(name="ps_dt", bufs=1, space="PSUM"))
        ps_db = ctx.enter_context(tc.tile_pool(name="ps_db", bufs=2, space="PSUM"))
        ps_eT = ctx.enter_context(tc.tile_pool(name="ps_eT", bufs=1, space="PSUM"))
        ps_w = ctx.enter_context(tc.tile_pool(name="ps_w", bufs=2, space="PSUM"))

        # ---------------- preamble ----------------
        ident = singles.tile([P, P], F32)
        make_identity(nc, ident)
        ident_r = singles.tile([P, P], F32R)
        nc.vector.tensor_copy(ident_r, ident)

        ek_sb = singles.tile([E, D], F32)
        nc.sync.dma_start(out=ek_sb, in_=ek[:, :])

        # 1/||ek||: square+accum, then exp(-0.5*ln(x))
        ek_sq = singles.tile([E, D], BF16)  # scratch, value unused
        ek_nsq = singles.tile([E, 1], F32)
        nc.scalar.activation(out=ek_sq, in_=ek_sb, func=AF.Square, accum_out=ek_nsq)
        ek_ln = singles.tile([E, 1], F32)
        nc.scalar.activation(out=ek_ln, in_=ek_nsq, func=AF.Ln)
        inv_en = singles.tile([E, 1], F32)
        nc.scalar.activation(out=inv_en, in_=ek_ln, func=AF.Exp, scale=-0.5)

        # float32r copy of ek for matmul2 rhs
        ek_r = singles.tile([E, D], F32R)
        nc.vector.tensor_copy(ek_r, ek_sb)

        # ek scaled by 1/||ek|| (float32r), transposed -> ekT_s [128, 16, 64] f32r
        ek_s = singles.tile([E, D], F32R)
        nc.vector.tensor_scalar_mul(ek_s, in0=ek_sb, scalar1=inv_en)
        ekT_s = singles.tile([P, D_CHUNKS, E], F32R)
        for c in range(D_CHUNKS):
            pst = ps_zt.tile([P, 512], F32R, tag="ps_zt")
            nc.tensor.transpose(
                pst[:, 0:E], ek_s[:, c * P:(c + 1) * P], ident_r[0:E, 0:E]
            )
            nc.scalar.copy(out=ekT_s[:, c, :], in_=pst[:, 0:E])

        # ---------------- main loop: groups of 4 tiles (512 tokens) ---------
        def load_group(g):
            """Issue the paired 2-tile (2 MiB) z loads for group g."""
            pairs = []
            for h in range(G // 2):
                r0 = (g * G + 2 * h) * P
                z2 = zpool.tile([P, 2, D], F32R, tag="z")
                nc.sync.dma_start(
                    out=z2,
                    in_=z[r0:r0 + 2 * P, :].bitcast(F32R).rearrange(
                        "(t p) d -> p t d", p=P
                    ),
                )
                pairs.append(z2)
            return pairs

        next_pairs = load_group(0)
        prev = None  # (dback, invs, g) of the previous group, consumed one group late

        def emit_tail(prevst, t):
            """similarity + softmax + matmul2 + stores for tile t of group prevst[2]."""
            dback, pinvs, pg = prevst
            i = pg * G + t
            r0 = i * P

            # sim = dots * (1/||z||)   (1/||ek|| folded into ekT_s)
            sim = smalls.tile([P, E], F32, tag="sim")
            nc.vector.tensor_scalar_mul(
                sim, in0=dback[:, t * E:(t + 1) * E], scalar1=pinvs[t]
            )
            nc.sync.dma_start(out=sim_out[r0:r0 + P, :], in_=sim)

            # softmax numerator/denominator (|sim|<=1: no max-subtract)
            e_t = smalls.tile([P, E], F32, tag="et")
            S = smalls.tile([P, 1], F32, tag="S")
            nc.scalar.activation(out=e_t, in_=sim, func=AF.Exp, accum_out=S)
            r_t = smalls.tile([P, 1], F32, tag="r")
            nc.vector.reciprocal(r_t, S)

            # transpose e -> eT [64, 128] (f32r on evict)
            pse = ps_eT.tile([E, P], F32, tag="ps_eT")
            nc.tensor.transpose(pse, e_t, ident)
            eT = smalls.tile([E, P], F32R, tag="eT")
            nc.vector.tensor_copy(eT, pse)

            # matmul2: w = e @ ek  [128, 2048] in 4 chunks of 512 (f32r)
            if t % 2 == 0:
                w_new = wpool.tile([P, 2, D], F32, tag="w")
                wtiles[0] = w_new
            w2 = wtiles[0]
            for j in range(4):
                wp = ps_w.tile([P, 512], F32, tag="ps_w")
                nc.tensor.matmul(
                    wp, eT, ek_r[:, j * 512:(j + 1) * 512],
                    start=True, stop=True,
                )
                nc.vector.tensor_scalar_mul(
                    w2[:, t % 2, j * 512:(j + 1) * 512], in0=wp, scalar1=r_t
                )
            if t % 2 == 1:
                # paired 2-tile (2 MiB) store
                nc.sync.dma_start(
                    out=w_out[r0 - P:r0 + P, :].rearrange(
                        "(t p) d -> p t d", p=P
                    ),
                    in_=w2,
                )

        wtiles = [None]
        for g in range(N_GROUPS + 1):
            if g < N_GROUPS:
                invs = []  # 1/||z|| per tile
                # zT for the whole group: [128, 16 chunks, 512 tokens] f32r
                zT = ztpool.tile([P, D_CHUNKS, G * P], F32R, tag="zT")
                zpairs = next_pairs
                if g + 1 < N_GROUPS:
                    next_pairs = load_group(g + 1)

            for t in range(G):
                if g < N_GROUPS:
                    i = g * G + t
                    z_t = zpairs[t // 2][:, t % 2, :]

                    # ||z||^2 -> 1/||z|| (stay in one ACT table set)
                    z_sq = singles.tile([P, D], BF16, tag="zsq_scratch")
                    z_nsq = smalls.tile([P, 1], F32, tag="znsq")
                    nc.scalar.activation(
                        out=z_sq, in_=z_t.bitcast(F32), func=AF.Square,
                        accum_out=z_nsq,
                    )
                    z_ln = smalls.tile([P, 1], F32, tag="zln")
                    nc.scalar.activation(out=z_ln, in_=z_nsq, func=AF.Ln)
                    inv_zn = smalls.tile([P, 1], F32, tag="invzn")
                    nc.scalar.activation(
                        out=inv_zn, in_=z_ln, func=AF.Exp, scale=-0.5
                    )
                    invs.append(inv_zn)

                    # transpose z tile -> zT[:, :, t*128:(t+1)*128]
                    for b in range(4):
                        pst = ps_zt.tile([P, 512], F32R, tag="ps_zt")
                        for j in range(4):
                            c = 4 * b + j
                            nc.tensor.transpose(
                                pst[:, j * P:(j + 1) * P],
                                z_t[:, c * P:(c + 1) * P],
                                ident_r,
                            )
                        # evict, alternating engines (ACT Copy = table filler)
                        dst = zT[:, 4 * b:4 * b + 4, t * P:(t + 1) * P]
                        if b % 2 == 0:
                            nc.scalar.copy(out=dst, in_=pst)
                        else:
                            nc.vector.tensor_copy(dst, pst)

                # interleave: tail of the same tile index in the previous group
                if prev is not None:
                    emit_tail(prev, t)

            if g < N_GROUPS:
                # matmul1 (flipped): dotsT[64, 512] += ekT_s_c.T @ zT_c
                dT = ps_dt.tile([E, G * P], F32, tag="dT")
                for c in range(D_CHUNKS):
                    nc.tensor.matmul(
                        dT, ekT_s[:, c, :], zT[:, c, :],
                        start=(c == 0), stop=(c == D_CHUNKS - 1),
                    )
                dT_sb = dtpool.tile([E, G * P], F32, tag="dT_sb")
                nc.vector.tensor_copy(dT_sb, dT)

                # transpose dotsT back to [128, 64] per tile (shared psum bank)
                dback = ps_db.tile([P, G * E], F32, tag="dback")
                for t in range(G):
                    nc.tensor.transpose(
                        dback[:, t * E:(t + 1) * E],
                        dT_sb[:, t * P:(t + 1) * P],
                        ident[0:E, 0:E],
                    )
                prev = (dback, invs, g)

    _split_multi_waits(nc)
    return nc


_NC = None


def _get_nc():
    global _NC
    if _NC is None:
        _NC = _build()
    return _NC


def kernel(z, expert_keys):
    z = np.asarray(z, dtype=np.float32)
    expert_keys = np.ascontiguousarray(np.asarray(expert_keys, dtype=np.float32))
    nc = _get_nc()
    in_maps = [
        {
            "z": np.ascontiguousarray(z[c * N_SHARD:(c + 1) * N_SHARD]),
            "expert_keys": expert_keys,
        }
        for c in range(N_CORES)
    ]
    res = run_bass_kernel_spmd(nc, in_maps, core_ids=list(range(N_CORES)))
    sim = np.concatenate([res.results[c]["similarity"] for c in range(N_CORES)], axis=0)
    w = np.concatenate([res.results[c]["weighted"] for c in range(N_CORES)], axis=0)
    return sim, w


# revision 19
# speedup vs baseline: 1.0466x; 1.0466x over previous
"""Cosine-similarity MoE routing kernel for Trainium2 (8 NeuronCores, SPMD).

reference:
    dots = z @ ek.T                      # [N, E]
    sim  = dots / max(||z|| * ||ek||, 1e-8)
    weighted = softmax(sim, -1) @ ek     # [N, D]
    returns (sim, weighted)

Sharding: data-parallel over tokens (N) across 8 cores; expert_keys replicated.

Design notes:
  - All matmuls in float32r (TF32-like, 1 cyc/col at N>=512; ~1.6e-4 rel err).
  - matmul1 flipped: lhsT = ekT_scaled chunk [128d, 64] stationary, rhs = zT
    [128d, 512 tokens] moving -> dotsT [64, 512] psum; one PSUM bank; 16-chunk
    accumulation; processed in G-tile groups.
  - z enters PE as float32r (bitcast, no cast cost); transposes at 1.5 cyc/col.
  - ScalarE stays in ONE table set (natural_log_exp_and_others): Square, Ln,
    Exp, Copy.  1/sqrt(x) = exp(-0.5*ln(x)); no Sqrt, no Reciprocal on ACT.
  - PSUM->SBUF evictions split between ScalarE (zT) and VectorE (weighted).
  - zT group buffer bufs=1: reuse is PE-serial (write-after-PE-read), free.
"""

from contextlib import ExitStack

import numpy as np

import concourse.bass as bass
import concourse.mybir as mybir
import concourse.tile as tile
from concourse.bass_utils import run_bass_kernel_spmd
from concourse.masks import make_identity

F32 = mybir.dt.float32
F32R = mybir.dt.float32r
BF16 = mybir.dt.bfloat16
AF = mybir.ActivationFunctionType

N_CORES = 8
N_FULL, D, E = 32768, 2048, 64
N_SHARD = N_FULL // N_CORES  # 4096
P = 128
G = 4                        # tiles per group (512 tokens)
N_GROUPS = N_SHARD // (G * P)
D_CHUNKS = D // P            # 16


def _split_multi_waits(nc, max_waits=1):
    """This walrus build allows very few sem-waits per instruction; split the
    extras into same-engine NOPs placed immediately before the instruction."""
    n = 0
    for func in nc.m.functions:
        for block in func.blocks:
            out = []
            for inst in block.instructions:
                si = inst.sync_info
                waits = list(si.on_wait) if (si is not None and si.on_wait) else []
                if len(waits) > max_waits:
                    head, keep = waits[:-max_waits], waits[-max_waits:]
                    for j, w in enumerate(head):
                        out.append(
                            mybir.InstNoOp(
                                name=f"{inst.name}_wsplit{j}",
                                engine=inst.engine,
                                sync_info=mybir.SyncInfo(on_wait=[w], on_update=[]),
                                bass_nofuse=True,
                            )
                        )
                        n += 1
                    si.on_wait = keep
                out.append(inst)
            block.instructions[:] = out
    return n


def _build():
    nc = bass.Bass(trn_type="TRN2")
    z = nc.dram_tensor("z", [N_SHARD, D], F32, kind="ExternalInput")
    ek = nc.dram_tensor("expert_keys", [E, D], F32, kind="ExternalInput")
    sim_out = nc.dram_tensor("similarity", [N_SHARD, E], F32, kind="ExternalOutput")
    w_out = nc.dram_tensor("weighted", [N_SHARD, D], F32, kind="ExternalOutput")

    with tile.TileContext(nc) as tc, ExitStack() as ctx:
        singles = ctx.enter_context(tc.tile_pool(name="singles", bufs=1))
        zpool = ctx.enter_context(tc.tile_pool(name="zpool", bufs=4))
        ztpool = ctx.enter_context(tc.tile_pool(name="ztpool", bufs=1))
        wpool = ctx.enter_context(tc.tile_pool(name="wpool", bufs=2))
        smalls = ctx.enter_context(tc.tile_pool(name="smalls", bufs=6))
        dtpool = ctx.enter_context(tc.tile_pool(name="dtpool", bufs=2))
        ps_zt = ctx.enter_context(tc.tile_pool(name="ps_zt", bufs=3, space="PSUM"))
        ps_dt = ctx.enter_context(tc.tile_pool(name="ps_dt", bufs=1, space="PSUM"))
        ps_db = ctx.enter_context(tc.tile_pool(name="ps_db", bufs=1, space="PSUM"))
        ps_w = ctx.enter_context(tc.tile_pool(name="ps_w", bufs=3, space="PSUM"))

        # ---------------- preamble ----------------
        ident = singles.tile([P, P], F32)
        make_identity(nc, ident)
        ident_r = singles.tile([P, P], F32R)
        nc.vector.tensor_copy(ident_r, ident)

        ek_sb = singles.tile([E, D], F32)
        nc.sync.dma_start(out=ek_sb, in_=ek[:, :])

        # 1/||ek||: square+accum, then exp(-0.5*ln(x))
        ek_sq = singles.tile([E, D], BF16)  # scratch, value unused
        ek_nsq = singles.tile([E, 1], F32)
        nc.scalar.activation(out=ek_sq, in_=ek_sb, func=AF.Square, accum_out=ek_nsq)
        ek_ln = singles.tile([E, 1], F32)
        nc.scalar.activation(out=ek_ln, in_=ek_nsq, func=AF.Ln)
        inv_en = singles.tile([E, 1], F32)
        nc.scalar.activation(out=inv_en, in_=ek_ln, func=AF.Exp, scale=-0.5)

        # float32r copy of ek for matmul2 rhs, duplicated onto both PE
        # row-halves so alternate tiles can use tile_position=(64,0) and their
        # LDWEIGHTS overlaps the other half's in-flight matmuls.
        ek_r = singles.tile([P, D], F32R)
        nc.vector.tensor_copy(ek_r[0:E, :], ek_sb)
        nc.vector.tensor_copy(ek_r[E:P, :], ek_sb)

        # ek scaled by 1/||ek|| (float32r), transposed -> ekT_s [128, 16, 64] f32r
        ek_s = singles.tile([E, D], F32R)
        nc.vector.tensor_scalar_mul(ek_s, in0=ek_sb, scalar1=inv_en)
        ekT_s = singles.tile([P, D_CHUNKS, E], F32R)
        for c in range(D_CHUNKS):
            pst = ps_zt.tile([P, 512], F32R, tag="ps_zt")
            nc.tensor.transpose(
                pst[:, 0:E], ek_s[:, c * P:(c + 1) * P], ident_r[0:E, 0:E]
            )
            nc.scalar.copy(out=ekT_s[:, c, :], in_=pst[:, 0:E])

        # ---------------- main loop: groups of G tiles ----------------------
        def load_group(g):
            """Issue the paired 2-tile (2 MiB) z loads for group g."""
            pairs = []
            for h in range(G // 2):
                r0 = (g * G + 2 * h) * P
                z2 = zpool.tile([P, 2, D], F32R, tag="z")
                nc.sync.dma_start(
                    out=z2,
                    in_=z[r0:r0 + 2 * P, :].bitcast(F32R).rearrange(
                        "(t p) d -> p t d", p=P
                    ),
                )
                pairs.append(z2)
            return pairs

        next_pairs = load_group(0)
        for g in range(N_GROUPS):
            invs = []  # 1/||z|| per tile
            # zT for the whole group: [128, 16 chunks, G*128 tokens] f32r
            zT = ztpool.tile([P, D_CHUNKS, G * P], F32R, tag="zT")

            zpairs = next_pairs
            if g + 1 < N_GROUPS:
                next_pairs = load_group(g + 1)

            for t in range(G):
                z_t = zpairs[t // 2][:, t % 2, :]

                # ||z||^2 -> 1/||z|| (stay in one ACT table set)
                z_sq = singles.tile([P, D], BF16, tag="zsq_scratch")
                z_nsq = smalls.tile([P, 1], F32, tag="znsq")
                nc.scalar.activation(
                    out=z_sq, in_=z_t.bitcast(F32), func=AF.Square, accum_out=z_nsq
                )
                z_ln = smalls.tile([P, 1], F32, tag="zln")
                nc.scalar.activation(out=z_ln, in_=z_nsq, func=AF.Ln)
                inv_zn = smalls.tile([P, 1], F32, tag="invzn")
                nc.scalar.activation(out=inv_zn, in_=z_ln, func=AF.Exp, scale=-0.5)
                invs.append(inv_zn)

                # transpose z tile -> zT[:, :, t*128:(t+1)*128]
                for b in range(4):
                    pst = ps_zt.tile([P, 512], F32R, tag="ps_zt")
                    for j in range(4):
                        c = 4 * b + j
                        nc.tensor.transpose(
                            pst[:, j * P:(j + 1) * P],
                            z_t[:, c * P:(c + 1) * P],
                            ident_r,
                        )
                    # evict, alternating engines (ACT Copy is a table filler)
                    dst = zT[:, 4 * b:4 * b + 4, t * P:(t + 1) * P]
                    if b % 2 == 0:
                        nc.scalar.copy(out=dst, in_=pst)
                    else:
                        nc.vector.tensor_copy(dst, pst)

            # matmul1 (flipped): dotsT[64, 512] += ekT_s_c.T @ zT_c
            dT = ps_dt.tile([E, G * P], F32, tag="dT")
            for c in range(D_CHUNKS):
                nc.tensor.matmul(
                    dT, ekT_s[:, c, :], zT[:, c, :],
                    start=(c == 0), stop=(c == D_CHUNKS - 1),
                )
            dT_sb = dtpool.tile([E, G * P], F32, tag="dT_sb")
            nc.scalar.copy(out=dT_sb, in_=dT)

            # transpose dotsT back to [128, 64] per tile (one shared psum bank)
            dback = ps_db.tile([P, G * E], F32, tag="dback")
            for t in range(G):
                nc.tensor.transpose(
                    dback[:, t * E:(t + 1) * E],
                    dT_sb[:, t * P:(t + 1) * P],
                    ident[0:E, 0:E],
                )

            w2 = None
            for t in range(G):
                r0 = (g * G + t) * P

                # sim = dots * (1/||z||)   (1/||ek|| folded into ekT_s)
                sim = smalls.tile([P, E], F32, tag="sim")
                nc.vector.tensor_scalar_mul(
                    sim, in0=dback[:, t * E:(t + 1) * E], scalar1=invs[t]
                )
                nc.sync.dma_start(out=sim_out[r0:r0 + P, :], in_=sim)

                # softmax numerator/denominator (|sim|<=1: no max-subtract).
                # exp lands in columns half*64..half*64+63 of a 128-wide tile,
                # so the transpose places experts on alternating PE row-halves
                # (the other half holds garbage, never read).
                half = t % 2
                e_t = smalls.tile([P, P], F32, tag="et")
                S = smalls.tile([P, 1], F32, tag="S")
                nc.scalar.activation(
                    out=e_t[:, half * E:(half + 1) * E], in_=sim, func=AF.Exp,
                    accum_out=S,
                )
                r_t = smalls.tile([P, 1], F32, tag="r")
                nc.vector.reciprocal(r_t, S)

                # transpose e -> eT; experts on partitions half*64..half*64+63
                pse = ps_w.tile([P, P], F32, tag="ps_w")
                nc.tensor.transpose(pse, e_t, ident)
                eT = smalls.tile([P, P], F32R, tag="eT")
                nc.vector.tensor_copy(
                    eT[half * E:(half + 1) * E, :], pse[half * E:(half + 1) * E, :]
                )

                # matmul2: w = e @ ek  [128, 2048] in 4 chunks of 512 (f32r)
                if t % 2 == 0:
                    w2 = wpool.tile([P, 2, D], F32, tag="w")
                for j in range(4):
                    wp = ps_w.tile([P, 512], F32, tag="ps_w")
                    nc.tensor.matmul(
                        wp,
                        eT[half * E:(half + 1) * E, :],
                        ek_r[half * E:(half + 1) * E, j * 512:(j + 1) * 512],
                        start=True, stop=True,
                        tile_position=(half * E, 0),
                    )
                    if j % 2 == 0:
                        nc.vector.tensor_scalar_mul(
                            w2[:, t % 2, j * 512:(j + 1) * 512], in0=wp, scalar1=r_t
                        )
                    else:
                        nc.scalar.activation(
                            out=w2[:, t % 2, j * 512:(j + 1) * 512], in_=wp,
                            func=AF.Identity, scale=r_t,
                        )
                if t % 2 == 1:
                    # paired 2-tile (2 MiB) store
                    nc.sync.dma_start(
                        out=w_out[r0 - P:r0 + P, :].rearrange(
                            "(t p) d -> p t d", p=P
                        ),
                        in_=w2,
                    )

    _split_multi_waits(nc)
    return nc


_NC = None


def _get_nc():
    global _NC
    if _NC is None:
        _NC = _build()
    return _NC


def kernel(z, expert_keys):
    z = np.asarray(z, dtype=np.float32)
    expert_keys = np.ascontiguousarray(np.asarray(expert_keys, dtype=np.float32))
    nc = _get_nc()
    in_maps = [
        {
            "z": np.ascontiguousarray(z[c * N_SHARD:(c + 1) * N_SHARD]),
            "expert_keys": expert_keys,
        }
        for c in range(N_CORES)
    ]
    res = run_bass_kernel_spmd(nc, in_maps, core_ids=list(range(N_CORES)))
    sim = np.concatenate([res.results[c]["similarity"] for c in range(N_CORES)], axis=0)
    w = np.concatenate([res.results[c]["weighted"] for c in range(N_CORES)], axis=0)
    return sim, w


# revision 20
# speedup vs baseline: 1.1083x; 1.0590x over previous
"""Cosine-similarity MoE routing kernel for Trainium2 (8 NeuronCores, SPMD).

reference:
    dots = z @ ek.T                      # [N, E]
    sim  = dots / max(||z|| * ||ek||, 1e-8)
    weighted = softmax(sim, -1) @ ek     # [N, D]
    returns (sim, weighted)

Sharding: data-parallel over tokens (N) across 8 cores; expert_keys replicated.

Design notes:
  - All matmuls in float32r (TF32-like, 1 cyc/col at N>=512; ~1.6e-4 rel err).
  - matmul1 flipped: lhsT = ekT_scaled chunk [128d, 64] stationary, rhs = zT
    [128d, 512 tokens] moving -> dotsT [64, 512] psum; one PSUM bank; 16-chunk
    accumulation; processed in G-tile groups.
  - z enters PE as float32r (bitcast, no cast cost); transposes at 1.5 cyc/col.
  - ScalarE stays in ONE table set (natural_log_exp_and_others): Square, Ln,
    Exp, Copy.  1/sqrt(x) = exp(-0.5*ln(x)); no Sqrt, no Reciprocal on ACT.
  - PSUM->SBUF evictions split between ScalarE (zT) and VectorE (weighted).
  - zT group buffer bufs=1: reuse is PE-serial (write-after-PE-read), free.
"""

from contextlib import ExitStack

import numpy as np

import concourse.bass as bass
import concourse.mybir as mybir
import concourse.tile as tile
from concourse.bass_utils import run_bass_kernel_spmd
from concourse.masks import make_identity

F32 = mybir.dt.float32
F32R = mybir.dt.float32r
BF16 = mybir.dt.bfloat16
AF = mybir.ActivationFunctionType

N_CORES = 8
N_FULL, D, E = 32768, 2048, 64
N_SHARD = N_FULL // N_CORES  # 4096
P = 128
G = 4                        # tiles per group (512 tokens)
N_GROUPS = N_SHARD // (G * P)
D_CHUNKS = D // P            # 16


def _split_multi_waits(nc, max_waits=1):
    """This walrus build allows very few sem-waits per instruction; split the
    extras into same-engine NOPs placed immediately before the instruction."""
    n = 0
    for func in nc.m.functions:
        for block in func.blocks:
            out = []
            for inst in block.instructions:
                si = inst.sync_info
                waits = list(si.on_wait) if (si is not None and si.on_wait) else []
                if len(waits) > max_waits:
                    head, keep = waits[:-max_waits], waits[-max_waits:]
                    for j, w in enumerate(head):
                        out.append(
                            mybir.InstNoOp(
                                name=f"{inst.name}_wsplit{j}",
                                engine=inst.engine,
                                sync_info=mybir.SyncInfo(on_wait=[w], on_update=[]),
                                bass_nofuse=True,
                            )
                        )
                        n += 1
                    si.on_wait = keep
                out.append(inst)
            block.instructions[:] = out
    return n


def _build():
    nc = bass.Bass(trn_type="TRN2")
    z = nc.dram_tensor("z", [N_SHARD, D], F32, kind="ExternalInput")
    ek = nc.dram_tensor("expert_keys", [E, D], F32, kind="ExternalInput")
    sim_out = nc.dram_tensor("similarity", [N_SHARD, E], F32, kind="ExternalOutput")
    w_out = nc.dram_tensor("weighted", [N_SHARD, D], F32, kind="ExternalOutput")

    with tile.TileContext(nc) as tc, ExitStack() as ctx:
        singles = ctx.enter_context(tc.tile_pool(name="singles", bufs=1))
        zpool = ctx.enter_context(tc.tile_pool(name="zpool", bufs=4))
        ztpool = ctx.enter_context(tc.tile_pool(name="ztpool", bufs=1))
        wpool = ctx.enter_context(tc.tile_pool(name="wpool", bufs=2))
        smalls = ctx.enter_context(tc.tile_pool(name="smalls", bufs=6))
        dtpool = ctx.enter_context(tc.tile_pool(name="dtpool", bufs=2))
        ps_zt = ctx.enter_context(tc.tile_pool(name="ps_zt", bufs=3, space="PSUM"))
        ps_dt = ctx.enter_context(tc.tile_pool(name="ps_dt", bufs=1, space="PSUM"))
        ps_db = ctx.enter_context(tc.tile_pool(name="ps_db", bufs=1, space="PSUM"))
        ps_w = ctx.enter_context(tc.tile_pool(name="ps_w", bufs=3, space="PSUM"))

        # ---------------- preamble ----------------
        ident = singles.tile([P, P], F32)
        make_identity(nc, ident)
        ident_r = singles.tile([P, P], F32R)
        nc.vector.tensor_copy(ident_r, ident)

        ek_sb = singles.tile([E, D], F32)
        nc.sync.dma_start(out=ek_sb, in_=ek[:, :])

        # 1/||ek||: square+accum, then exp(-0.5*ln(x))
        ek_sq = singles.tile([E, D], BF16)  # scratch, value unused
        ek_nsq = singles.tile([E, 1], F32)
        nc.scalar.activation(out=ek_sq, in_=ek_sb, func=AF.Square, accum_out=ek_nsq)
        ek_ln = singles.tile([E, 1], F32)
        nc.scalar.activation(out=ek_ln, in_=ek_nsq, func=AF.Ln)
        inv_en = singles.tile([E, 1], F32)
        nc.scalar.activation(out=inv_en, in_=ek_ln, func=AF.Exp, scale=-0.5)

        # float32r copy of ek for matmul2 rhs, duplicated onto both PE
        # row-halves so alternate tiles can use tile_position=(64,0) and their
        # LDWEIGHTS overlaps the other half's in-flight matmuls.
        ek_r = singles.tile([P, D], F32R)
        nc.vector.tensor_copy(ek_r[0:E, :], ek_sb)
        nc.vector.tensor_copy(ek_r[E:P, :], ek_sb)

        # ek scaled by 1/||ek|| (float32r), transposed -> ekT_s [128, 16, 64] f32r
        ek_s = singles.tile([E, D], F32R)
        nc.vector.tensor_scalar_mul(ek_s, in0=ek_sb, scalar1=inv_en)
        ekT_s = singles.tile([P, D_CHUNKS, E], F32R)
        for c in range(D_CHUNKS):
            pst = ps_zt.tile([P, 512], F32R, tag="ps_zt")
            nc.tensor.transpose(
                pst[:, 0:E], ek_s[:, c * P:(c + 1) * P], ident_r[0:E, 0:E]
            )
            nc.scalar.copy(out=ekT_s[:, c, :], in_=pst[:, 0:E])

        # ---------------- main loop: groups of G tiles ----------------------
        def load_group(g):
            """Issue the paired 2-tile (2 MiB) z loads for group g."""
            pairs = []
            for h in range(G // 2):
                r0 = (g * G + 2 * h) * P
                z2 = zpool.tile([P, 2, D], F32R, tag="z")
                nc.sync.dma_start(
                    out=z2,
                    in_=z[r0:r0 + 2 * P, :].bitcast(F32R).rearrange(
                        "(t p) d -> p t d", p=P
                    ),
                )
                pairs.append(z2)
            return pairs

        next_pairs = load_group(0)
        for g in range(N_GROUPS):
            invs = []  # 1/||z|| per tile
            # zT for the whole group: [128, 16 chunks, G*128 tokens] f32r
            zT = ztpool.tile([P, D_CHUNKS, G * P], F32R, tag="zT")

            zpairs = next_pairs
            if g + 1 < N_GROUPS:
                next_pairs = load_group(g + 1)

            # norms for each tile first (ACT-only, overlaps transposes)
            for t in range(G):
                z_t = zpairs[t // 2][:, t % 2, :]
                z_sq = singles.tile([P, D], BF16, tag="zsq_scratch")
                z_nsq = smalls.tile([P, 1], F32, tag="znsq")
                nc.scalar.activation(
                    out=z_sq, in_=z_t.bitcast(F32), func=AF.Square, accum_out=z_nsq
                )
                z_ln = smalls.tile([P, 1], F32, tag="zln")
                nc.scalar.activation(out=z_ln, in_=z_nsq, func=AF.Ln)
                inv_zn = smalls.tile([P, 1], F32, tag="invzn")
                nc.scalar.activation(out=inv_zn, in_=z_ln, func=AF.Exp, scale=-0.5)
                invs.append(inv_zn)

            # chunk-major transposes with mm1 partials interleaved: after each
            # chunk-group b is transposed for all G tiles, its 4 matmul1
            # accumulations run -> dense MAC activity throughout the group.
            dT = ps_dt.tile([E, G * P], F32, tag="dT")
            for b in range(4):
                for t in range(G):
                    z_t = zpairs[t // 2][:, t % 2, :]
                    pst = ps_zt.tile([P, 512], F32R, tag="ps_zt")
                    for j in range(4):
                        c = 4 * b + j
                        nc.tensor.transpose(
                            pst[:, j * P:(j + 1) * P],
                            z_t[:, c * P:(c + 1) * P],
                            ident_r,
                        )
                    # evict, alternating engines (ACT Copy is a table filler)
                    dst = zT[:, 4 * b:4 * b + 4, t * P:(t + 1) * P]
                    if t % 2 == 0:
                        nc.scalar.copy(out=dst, in_=pst)
                    else:
                        nc.vector.tensor_copy(dst, pst)
                for j in range(4):
                    c = 4 * b + j
                    nc.tensor.matmul(
                        dT, ekT_s[:, c, :], zT[:, c, :],
                        start=(c == 0), stop=(c == D_CHUNKS - 1),
                    )
            dT_sb = dtpool.tile([E, G * P], F32, tag="dT_sb")
            nc.scalar.copy(out=dT_sb, in_=dT)

            # transpose dotsT back to [128, 64] per tile (one shared psum bank)
            dback = ps_db.tile([P, G * E], F32, tag="dback")
            for t in range(G):
                nc.tensor.transpose(
                    dback[:, t * E:(t + 1) * E],
                    dT_sb[:, t * P:(t + 1) * P],
                    ident[0:E, 0:E],
                )

            w2 = None
            for t in range(G):
                r0 = (g * G + t) * P

                # sim = dots * (1/||z||)   (1/||ek|| folded into ekT_s)
                sim = smalls.tile([P, E], F32, tag="sim")
                nc.vector.tensor_scalar_mul(
                    sim, in0=dback[:, t * E:(t + 1) * E], scalar1=invs[t]
                )
                nc.sync.dma_start(out=sim_out[r0:r0 + P, :], in_=sim)

                # softmax numerator/denominator (|sim|<=1: no max-subtract)
                e_t = smalls.tile([P, E], F32, tag="et")
                S = smalls.tile([P, 1], F32, tag="S")
                nc.scalar.activation(out=e_t, in_=sim, func=AF.Exp, accum_out=S)
                r_t = smalls.tile([P, 1], F32, tag="r")
                nc.vector.reciprocal(r_t, S)

                # transpose e -> eT [64, 128]; psum slot shared with ps_w pool
                pse = ps_w.tile([E, P], F32, tag="ps_w")
                nc.tensor.transpose(pse, e_t, ident)
                eT = smalls.tile([E, P], F32R, tag="eT")
                nc.vector.tensor_copy(eT, pse)

                # matmul2: w = e @ ek  [128, 2048] in 4 chunks of 512 (f32r)
                if t % 2 == 0:
                    w2 = wpool.tile([P, 2, D], F32, tag="w")
                for j in range(4):
                    wp = ps_w.tile([P, 512], F32, tag="ps_w")
                    nc.tensor.matmul(
                        wp, eT, ek_r[0:E, j * 512:(j + 1) * 512],
                        start=True, stop=True,
                    )
                    nc.vector.tensor_scalar_mul(
                        w2[:, t % 2, j * 512:(j + 1) * 512], in0=wp, scalar1=r_t
                    )
                if t % 2 == 1:
                    # paired 2-tile (2 MiB) store
                    nc.sync.dma_start(
                        out=w_out[r0 - P:r0 + P, :].rearrange(
                            "(t p) d -> p t d", p=P
                        ),
                        in_=w2,
                    )

    _split_multi_waits(nc)
    return nc


_NC = None


def _get_nc():
    global _NC
    if _NC is None:
        _NC = _build()
    return _NC


def kernel(z, expert_keys):
    z = np.asarray(z, dtype=np.float32)
    expert_keys = np.ascontiguousarray(np.asarray(expert_keys, dtype=np.float32))
    nc = _get_nc()
    in_maps = [
        {
            "z": np.ascontiguousarray(z[c * N_SHARD:(c + 1) * N_SHARD]),
            "expert_keys": expert_keys,
        }
        for c in range(N_CORES)
    ]
    res = run_bass_kernel_spmd(nc, in_maps, core_ids=list(range(N_CORES)))
    sim = np.concatenate([res.results[c]["similarity"] for c in range(N_CORES)], axis=0)
    w = np.concatenate([res.results[c]["weighted"] for c in range(N_CORES)], axis=0)
    return sim, w
